# revision 26
# baseline (speedup 1.0000x reference)
"""Distributed causal multi-head attention for TRN2 (8 NeuronCores).

Problem: x[2,2048,1024], w_qkv[1024,16,192], w_out[16,64,1024] (biases zero).
Sharding: 2 batch groups x 4-way tensor-parallel over heads (4 heads/core).
Per core: QKV projection fused chunk-wise into causal flash-style attention
(attention for q-chunk r only needs x-chunks 0..r), 2-head PE-array packing
(row-split scores, col-split AV). The head-parallel reduction runs as a
chunked bf16 AllGather of attention outputs (1MB/core vs 4MB for the
all-reduce formulation); each core then output-projects the gathered
attention against its own 256-column slice of w_out, so the final output is
column-disjoint per core and host-side unsharding is a pure gather (+cast).

bf16 matmuls throughout (fp32 matmul is 2-pass on TRN2 = 1.9x slower);
softmax denominator via a ones-matmul that also broadcasts across
partitions; exp for both heads of a pair in one ACT instruction over a
two-bank PSUM tile; causal masks are precomputed tiles applied on DVE.
"""

import numpy as np

BS, S_FULL, D, H = 2, 2048, 1024, 16
DH = 64
P = 128
HL = 4              # heads per core
QCW = 512           # q-chunk width
NCORE = 8
GROUPS = [[0, 1, 2, 3], [4, 5, 6, 7]]
OSL = D // len(GROUPS[0])   # 256 output columns per core

_CACHE = {}


def build_graph(S=S_FULL):
    """Build the SPMD single-core graph (same on all 8 cores)."""
    import concourse.bacc as bacc
    import concourse.mybir as mybir
    import concourse.tile as tile
    from concourse.tile_rust import add_dep_helper

    F32 = mybir.dt.float32
    BF16 = mybir.dt.bfloat16
    Act = mybir.ActivationFunctionType
    Alu = mybir.AluOpType

    NDT = D // P                 # 8 d-tiles (contraction of qkv proj)
    NMC = S // QCW               # m-chunks of x / q-chunks
    NQC = NMC
    COT = (HL * DH) // P         # 2 c-tiles of the local attnT
    NFT = 2 * HL * DH // P       # 4 qk feature tiles
    GW = len(GROUPS[0])
    CFT = GW * COT               # 8 c-tiles of the gathered attnT

    nc = bacc.Bacc("TRN2", target_bir_lowering=False, debug=False,
                   num_devices=NCORE)

    xt_ext = nc.dram_tensor("xt", [P, NDT, S], F32, kind="ExternalInput")
    wqk_ext = nc.dram_tensor("wqk", [P, NDT, 2 * HL * DH], F32, kind="ExternalInput")
    wv_ext = nc.dram_tensor("wv", [P, NDT, HL * DH], F32, kind="ExternalInput")
    wout_ext = nc.dram_tensor("wout", [P, CFT, OSL], F32, kind="ExternalInput")
    bqk_ext = nc.dram_tensor("bqk", [NFT, P], F32, kind="ExternalInput")
    bv_ext = nc.dram_tensor("bv", [1, HL * DH], F32, kind="ExternalInput")
    bout_ext = nc.dram_tensor("bout", [1, OSL], F32, kind="ExternalInput")
    out_ext = nc.dram_tensor("out", [S, OSL], BF16, kind="ExternalOutput")

    with tile.TileContext(nc) as tc:
        with (
            tc.tile_pool(name="persist", bufs=1) as pp,
            tc.tile_pool(name="xchunk", bufs=1) as xp,
            tc.tile_pool(name="pt", bufs=6) as ptp,
            tc.tile_pool(name="recip", bufs=2) as rcp,
            tc.tile_pool(name="af", bufs=2) as afp,
            tc.tile_pool(name="outsb", bufs=4) as osp,
            tc.tile_pool(name="ps", bufs=1, space="PSUM") as ps,
            tc.tile_pool(name="dram", bufs=1, space="DRAM") as dp,
        ):
            # ---- persistent SBUF tensors ----
            wqk_sb = pp.tile([P, NDT * 512], BF16, name="wqk_sb")
            wv_sb = pp.tile([P, NDT * 256], BF16, name="wv_sb")
            wout_sb = pp.tile([P, CFT * OSL], BF16, name="wout_sb")
            bqk_sb = pp.tile([P, NFT], F32, name="bqk_sb")
            bv_row = pp.tile([1, 256], F32, name="bv_row")
            bvb_sb = pp.tile([P, 256], F32, name="bvb_sb")
            bob_row = pp.tile([1, OSL], F32, name="bob_row")
            bob_sb = pp.tile([P, OSL], F32, name="bob_sb")
            ones_sb = pp.tile([P, DH], BF16, name="ones_sb")
            warm_sb = pp.tile([4, DH], BF16, name="warm_sb")
            qkT = [pp.tile([P, S], BF16, name=f"qkT{ft}") for ft in range(NFT)]
            v_sb = pp.tile([P, (S // P) * 256], BF16, name="v_sb")
            attnT = [pp.tile([P, S], BF16, name=f"attnT{ct}") for ct in range(COT)]
            masks = [pp.tile([P, 2 * QCW], BF16, name=f"mask{j}")
                     for j in range(QCW // P)]

            # ---- DRAM bounce buffers for the AllGathers (bf16),
            # one per (round, head-pair) so each fires mid-round ----
            ag_in = [[dp.tile([P, QCW], BF16, name=f"ag_in{r}_{h}")
                      for h in range(2)] for r in range(NQC)]
            ag_out = [[dp.tile([GW * P, QCW], BF16, name=f"ag_out{r}_{h}")
                       for h in range(2)] for r in range(NQC)]
            warm_in = dp.tile([4, DH], BF16, name="warm_in")
            warm_out = dp.tile([16, DH], BF16, name="warm_out")

            # ---- loads (f32 -> bf16 cast during SWDGE DMA), criticals first
            for ft in range(NFT):
                nc.sync.dma_start(out=bqk_sb[:, ft:ft + 1],
                                  in_=bqk_ext[ft:ft + 1, :].rearrange("o p -> p o"))
            nc.sync.dma_start(out=bv_row[:], in_=bv_ext[:])
            nc.sync.dma_start(out=bob_row[:], in_=bout_ext[:])
            nc.vector.memset(ones_sb[:], 1.0)
            nc.vector.memset(warm_sb[:], 1.0)
            # preload the ACT exp table set before attention needs it
            nc.scalar.activation(warm_sb[0:4, 0:16], warm_sb[0:4, 0:16],
                                 Act.Exp)
            nc.sync.dma_start(out=warm_in[:], in_=warm_sb[:])
            for j in range(QCW // P):
                nc.vector.memset(masks[j][:], 1.0)

            # warm up the collective engine first (cold-start is ~25us and
            # the first trigger also absorbs cross-core start skew)
            warm_cc = nc.gpsimd.collective_compute(
                "AllGather", Alu.bypass, replica_groups=GROUPS,
                ins=[warm_in[:]], outs=[warm_out[:]])
            # everything round 0 needs comes first: wqk, xch0, wv, biases, masks
            xchs = [xp.tile([P, NDT * QCW], BF16, name=f"xch{mc}", tag=f"x{mc}")
                    for mc in range(NMC)]
            hd = NDT // 2
            for half in range(2):
                ds = slice(half * hd, (half + 1) * hd)
                dma_w = nc.gpsimd.dma_start(
                    out=wqk_sb[:, half * hd * 512:(half + 1) * hd * 512]
                        .rearrange("p (d f) -> p d f", d=hd),
                    in_=wqk_ext[:, ds])
                if half == 0:
                    # keep the scheduler from hoisting the big loads ahead
                    # of the collective warmup trigger on the gpsimd queue
                    add_dep_helper(dma_w.ins, warm_cc.ins, sync=False,
                                   reason="warmup cc first")
                nc.gpsimd.dma_start(
                    out=xchs[0][:, half * hd * QCW:(half + 1) * hd * QCW]
                        .rearrange("p (d m) -> p d m", d=hd),
                    in_=xt_ext[:, ds, 0:QCW])
            nc.gpsimd.dma_start(
                out=wv_sb[:].rearrange("p (d f) -> p d f", d=NDT),
                in_=wv_ext[:])
            nc.gpsimd.partition_broadcast(bvb_sb[:], bv_row[:])
            nc.gpsimd.partition_broadcast(bob_sb[:], bob_row[:])
            for j in range(QCW // P):
                nc.gpsimd.affine_select(
                    masks[j][:].rearrange("p (s w) -> p s w", s=2),
                    masks[j][:].rearrange("p (s w) -> p s w", s=2),
                    pattern=[[0, 2], [1, QCW]], compare_op=Alu.is_ge,
                    fill=0.0, base=-j * P, channel_multiplier=-1)
            for mc in range(1, NMC):
                nc.gpsimd.dma_start(
                    out=xchs[mc][:].rearrange("p (d m) -> p d m", d=NDT),
                    in_=xt_ext[:, :, mc * QCW:(mc + 1) * QCW])
                if mc == 1:
                    nc.gpsimd.dma_start(
                        out=wout_sb[:].rearrange("p (c f) -> p c f", c=CFT),
                        in_=wout_ext[:])

            # ---- projection work units (one x-chunk = 4 qk + 4 v units) ----
            def do_qk(mc, ft):
                xch = xchs[mc]
                pqk = ps.tile([P, 512], F32, name="pqk", tag="pv", bufs=2)
                for d in range(NDT):
                    nc.tensor.matmul(
                        pqk[:],
                        wqk_sb[:, d * 512 + ft * P:d * 512 + (ft + 1) * P],
                        xch[:, d * QCW:(d + 1) * QCW],
                        start=(d == 0), stop=(d == NDT - 1))
                nc.vector.tensor_scalar_add(
                    qkT[ft][:, mc * QCW:(mc + 1) * QCW], pqk[:],
                    bqk_sb[:, ft:ft + 1])

            def do_v(mc, mt):
                xch = xchs[mc]
                gmt = mc * (QCW // P) + mt
                pv = ps.tile([P, 256], F32, name="pv", tag="pv", bufs=2)
                for d in range(NDT):
                    nc.tensor.matmul(
                        pv[:],
                        xch[:, d * QCW + mt * P:d * QCW + (mt + 1) * P],
                        wv_sb[:, d * 256:(d + 1) * 256],
                        start=(d == 0), stop=(d == NDT - 1))
                nc.vector.tensor_add(v_sb[:, gmt * 256:(gmt + 1) * 256],
                                     pv[:], bvb_sb[:])

            # out-projection of the gathered attention for q-chunk rr:
            # out[q, osl] = sum_c attn_full[q, c] * w_out[c, osl]
            # (c rows arrive pair-split; w_out rows are host-permuted to match)
            af_tiles = {}

            def load_af(rr, h):
                af = afp.tile([P, GW * QCW], BF16, name=f"af{rr}_{h}",
                              tag="af")
                for ct in range(GW):
                    nc.sync.dma_start(
                        out=af[:, ct * QCW:(ct + 1) * QCW],
                        in_=ag_out[rr][h][ct * P:(ct + 1) * P, :])
                af_tiles[(rr, h)] = af

            def do_outproj(rr, qi, pin=None):
                po = ps.tile([P, OSL], F32, name="po", tag="pv", bufs=2)
                for h in range(2):
                    af = af_tiles[(rr, h)]
                    for ct in range(GW):
                        mm = nc.tensor.matmul(
                            po[:],
                            af[:, ct * QCW + qi * P:ct * QCW + (qi + 1) * P],
                            wout_sb[:, (h * GW + ct) * OSL:(h * GW + ct + 1) * OSL],
                            start=(h == 0 and ct == 0),
                            stop=(h == 1 and ct == GW - 1))
                        if pin is not None and h == 0 and ct == 0:
                            # ordering-only pin: keep the scheduler from
                            # hoisting this ahead of the current round's
                            # attention (it would stall PE on the AllGather)
                            add_dep_helper(mm.ins, pin.ins, sync=False,
                                           reason="outproj after attention")
                outsb = osp.tile([P, OSL], BF16, name="outsb", tag="ot")
                nc.vector.tensor_add(outsb[:], po[:], bob_sb[:])
                nc.sync.dma_start(
                    out=out_ext[rr * QCW + qi * P:rr * QCW + (qi + 1) * P, :],
                    in_=outsb[:])

            def proj_units(mc):
                return ([(do_qk, mc, ft) for ft in range(NFT)] +
                        [(do_v, mc, mt) for mt in range(QCW // P)])

            # chunk 0 projection up front
            for fn, a1, a2 in proj_units(0):
                fn(a1, a2)

            # ---- fused rounds ----
            for r in range(NQC):
                units = proj_units(r + 1) if r + 1 < NMC else []
                # out-projection of chunk r-1 (its AllGathers have landed),
                # paced into the last quarter of this round
                ounits = []
                if r >= 1:
                    load_af(r - 1, 0)
                    load_af(r - 1, 1)
                    ounits = [(do_outproj, r - 1, qi)
                              for qi in range(QCW // P)]
                ui = 0
                oi = 0
                last_mm = [None]
                nkt = (r + 1) * (QCW // P)
                steps_total = 2 * nkt
                step = 0
                q0 = r * QCW
                for pr in range(HL // 2):        # head pairs (2pr, 2pr+1)
                    qt_t = qkT[pr]
                    kt_t = qkT[2 + pr]
                    av = ps.tile([P, QCW], F32, name="av", tag="av", bufs=1)
                    den = ps.tile([P, QCW], F32, name="den", tag="den", bufs=1)

                    def av_den(kt, pt2, a, first, last):
                        for sub in (0, 1):
                            h = 2 * pr + sub
                            nc.tensor.matmul(
                                av[sub * DH:(sub + 1) * DH, a:QCW],
                                v_sb[:, kt * 256 + h * DH:kt * 256 + (h + 1) * DH],
                                pt2[:, sub * QCW + a:(sub + 1) * QCW],
                                start=first, stop=last, skip_group_check=True)
                        for sub in (0, 1):
                            nc.tensor.matmul(
                                den[sub * DH:(sub + 1) * DH, a:QCW],
                                ones_sb[:],
                                pt2[:, sub * QCW + a:(sub + 1) * QCW],
                                start=first, stop=last, skip_group_check=True)

                    pending = []
                    for kt in range(nkt):
                        off = max(0, (kt - 4 * r) * P)
                        a = off
                        s2 = ps.tile([P, 2 * QCW], F32, name="s2", tag="s",
                                     bufs=2)
                        for sub in (0, 1):
                            bp = sub * DH
                            last_mm[0] = nc.tensor.matmul(
                                s2[:, sub * QCW + a:(sub + 1) * QCW],
                                kt_t[bp:bp + DH, kt * P:(kt + 1) * P],
                                qt_t[bp:bp + DH, q0 + a:q0 + QCW],
                                start=True, stop=True)
                        pt2 = ptp.tile([P, 2 * QCW], BF16, name="pt2", tag="pt")
                        s2v = s2[:].rearrange("p (s w) -> p s w", s=2)
                        pt2v = pt2[:].rearrange("p (s w) -> p s w", s=2)
                        nc.scalar.activation(pt2v[:, :, a:QCW], s2v[:, :, a:QCW],
                                             Act.Exp)
                        if kt >= 4 * r:
                            j = kt - 4 * r
                            wlen = min(off + P, QCW) - a
                            mv = masks[j][:].rearrange("p (s w) -> p s w", s=2)
                            nc.vector.tensor_mul(pt2v[:, :, a:a + wlen],
                                                 pt2v[:, :, a:a + wlen],
                                                 mv[:, :, a:a + wlen])
                        # pace projection units across the round, out-proj
                        # units into its last quarter
                        if units and ui < len(units) and \
                                step * len(units) >= ui * steps_total:
                            fn, a1, a2 = units[ui]
                            fn(a1, a2)
                            ui += 1
                        q3 = 3 * steps_total // 4
                        if ounits and oi < len(ounits) and step >= q3 and \
                                (step - q3) * len(ounits) >= \
                                oi * (steps_total - q3):
                            fn, a1, a2 = ounits[oi]
                            fn(a1, a2, pin=last_mm[0])
                            oi += 1
                        pending.append((kt, pt2, a))
                        if len(pending) > 2:
                            pv_ = pending.pop(0)
                            av_den(*pv_, first=(pv_[0] == 0), last=False)
                        step += 1
                    while pending:
                        pv_ = pending.pop(0)
                        av_den(*pv_, first=(pv_[0] == 0),
                               last=(not pending))

                    recip = rcp.tile([P, QCW], F32, name="recip", tag="rc")
                    nc.vector.reciprocal_approx_fast(recip[:], den[:])
                    nc.vector.tensor_mul(attnT[pr][:, q0:q0 + QCW], av[:],
                                         recip[:])
                    # ship this pair's attention and AllGather it right away
                    nc.scalar.dma_start(out=ag_in[r][pr][:],
                                        in_=attnT[pr][:, q0:q0 + QCW])
                    nc.gpsimd.collective_compute(
                        "AllGather", Alu.bypass, replica_groups=GROUPS,
                        ins=[ag_in[r][pr][:]], outs=[ag_out[r][pr][:]])
                while ui < len(units):
                    fn, a1, a2 = units[ui]
                    fn(a1, a2)
                    ui += 1
                while oi < len(ounits):
                    fn, a1, a2 = ounits[oi]
                    fn(a1, a2)
                    oi += 1

            # final round's out-projection (tail)
            load_af(NQC - 1, 0)
            load_af(NQC - 1, 1)
            for qi in range(QCW // P):
                do_outproj(NQC - 1, qi)

    nc.compile()
    return nc


def shard_inputs(x, w_qkv, b_qkv, w_out, b_out, S=S_FULL):
    """Host-side sharding: per-core input dicts (pure layout work)."""
    scale = np.float32(DH ** -0.5)
    x = np.asarray(x, dtype=np.float32)
    w_qkv = np.asarray(w_qkv, dtype=np.float32)
    b_qkv = np.asarray(b_qkv, dtype=np.float32)
    w_out = np.asarray(w_out, dtype=np.float32)
    b_out = np.asarray(b_out, dtype=np.float32)
    NDT = D // P
    CFT = H * DH // P
    in_maps = []
    for c in range(NCORE):
        g, r = c // 4, c % 4
        hs = slice(HL * r, HL * (r + 1))
        osl = slice(OSL * r, OSL * (r + 1))
        xt = np.ascontiguousarray(
            x[g, :S].T.reshape(NDT, P, S).transpose(1, 0, 2))
        wq = (w_qkv[:, hs, 0:DH] * scale).reshape(D, HL * DH)
        wk = w_qkv[:, hs, DH:2 * DH].reshape(D, HL * DH)
        wqk = np.ascontiguousarray(
            np.concatenate([wq, wk], axis=1).reshape(NDT, P, 2 * HL * DH)
            .transpose(1, 0, 2))
        wv = np.ascontiguousarray(
            w_qkv[:, hs, 2 * DH:3 * DH].reshape(D, HL * DH)
            .reshape(NDT, P, HL * DH).transpose(1, 0, 2))
        head_order = ([4 * rr + j for rr in range(4) for j in (0, 1)] +
                      [4 * rr + j for rr in range(4) for j in (2, 3)])
        wo = np.ascontiguousarray(
            w_out[head_order].reshape(H * DH, D)[:, osl]
            .reshape(CFT, P, OSL).transpose(1, 0, 2))
        bq = (b_qkv[hs, 0:DH] * scale).reshape(HL * DH)
        bk = b_qkv[hs, DH:2 * DH].reshape(HL * DH)
        bqk = np.concatenate([bq, bk]).reshape(2 * HL * DH // P, P)
        bv = b_qkv[hs, 2 * DH:3 * DH].reshape(1, HL * DH)
        bout = b_out[osl].reshape(1, OSL)
        in_maps.append({
            "xt": np.ascontiguousarray(xt),
            "wqk": wqk, "wv": wv, "wout": wo,
            "bqk": np.ascontiguousarray(bqk),
            "bv": np.ascontiguousarray(bv),
            "bout": np.ascontiguousarray(bout),
        })
    return in_maps


def unshard_output(results, S=S_FULL):
    """Pure gather of per-core column slices into the full output (+cast)."""
    out = np.empty((BS, S, D), dtype=np.float32)
    for c in range(NCORE):
        g, r = c // 4, c % 4
        o = np.asarray(results[c]["out"]).astype(np.float32)
        out[g, :, OSL * r:OSL * (r + 1)] = o
    return out


def kernel(x, w_qkv, b_qkv, w_out, b_out, trace=False):
    from concourse.bass_utils import run_bass_kernel_spmd
    if "nc" not in _CACHE:
        _CACHE["nc"] = build_graph()
    nc = _CACHE["nc"]
    in_maps = shard_inputs(x, w_qkv, b_qkv, w_out, b_out)
    res = run_bass_kernel_spmd(nc, in_maps, core_ids=list(range(NCORE)),
                               trace=trace)
    _CACHE["last_results"] = res
    return unshard_output(res.results)


# revision 27
# speedup vs baseline: 1.0198x; 1.0198x over previous
"""Distributed causal multi-head attention for TRN2 (8 NeuronCores).

Problem: x[2,2048,1024], w_qkv[1024,16,192], w_out[16,64,1024] (biases zero).
Sharding: 2 batch groups x 4-way tensor-parallel over heads (4 heads/core).
Per core: QKV projection fused chunk-wise into causal flash-style attention
(attention for q-chunk r only needs x-chunks 0..r), 2-head PE-array packing
(row-split scores, col-split AV). The head-parallel reduction runs as a
chunked bf16 AllGather of attention outputs (1MB/core vs 4MB for the
all-reduce formulation); each core then output-projects the gathered
attention against its own 256-column slice of w_out, so the final output is
column-disjoint per core and host-side unsharding is a pure gather (+cast).

bf16 matmuls throughout (fp32 matmul is 2-pass on TRN2 = 1.9x slower);
softmax denominator via a ones-matmul that also broadcasts across
partitions; exp for both heads of a pair in one ACT instruction over a
two-bank PSUM tile; causal masks are precomputed tiles applied on DVE.
"""

import numpy as np

BS, S_FULL, D, H = 2, 2048, 1024, 16
DH = 64
P = 128
HL = 4              # heads per core
QCW = 512           # q-chunk width
NCORE = 8
GROUPS = [[0, 1, 2, 3], [4, 5, 6, 7]]
OSL = D // len(GROUPS[0])   # 256 output columns per core

_CACHE = {}


def build_graph(S=S_FULL):
    """Build the SPMD single-core graph (same on all 8 cores)."""
    import concourse.bacc as bacc
    import concourse.mybir as mybir
    import concourse.tile as tile
    from concourse.tile_rust import add_dep_helper

    F32 = mybir.dt.float32
    BF16 = mybir.dt.bfloat16
    Act = mybir.ActivationFunctionType
    Alu = mybir.AluOpType

    NDT = D // P                 # 8 d-tiles (contraction of qkv proj)
    NMC = S // QCW               # m-chunks of x / q-chunks
    NQC = NMC
    COT = (HL * DH) // P         # 2 c-tiles of the local attnT
    NFT = 2 * HL * DH // P       # 4 qk feature tiles
    GW = len(GROUPS[0])
    CFT = GW * COT               # 8 c-tiles of the gathered attnT

    nc = bacc.Bacc("TRN2", target_bir_lowering=False, debug=False,
                   num_devices=NCORE)

    xt_ext = nc.dram_tensor("xt", [P, NDT, S], F32, kind="ExternalInput")
    wqk_ext = nc.dram_tensor("wqk", [P, NDT, 2 * HL * DH], F32, kind="ExternalInput")
    wv_ext = nc.dram_tensor("wv", [P, NDT, HL * DH], F32, kind="ExternalInput")
    wout_ext = nc.dram_tensor("wout", [P, CFT, OSL], F32, kind="ExternalInput")
    bqk_ext = nc.dram_tensor("bqk", [NFT, P], F32, kind="ExternalInput")
    bv_ext = nc.dram_tensor("bv", [1, HL * DH], F32, kind="ExternalInput")
    bout_ext = nc.dram_tensor("bout", [1, OSL], F32, kind="ExternalInput")
    out_ext = nc.dram_tensor("out", [S, OSL], BF16, kind="ExternalOutput")

    with tile.TileContext(nc) as tc:
        with (
            tc.tile_pool(name="persist", bufs=1) as pp,
            tc.tile_pool(name="xchunk", bufs=1) as xp,
            tc.tile_pool(name="pt", bufs=6) as ptp,
            tc.tile_pool(name="recip", bufs=2) as rcp,
            tc.tile_pool(name="af", bufs=2) as afp,
            tc.tile_pool(name="outsb", bufs=4) as osp,
            tc.tile_pool(name="ps", bufs=1, space="PSUM") as ps,
            tc.tile_pool(name="dram", bufs=1, space="DRAM") as dp,
        ):
            # ---- persistent SBUF tensors ----
            wqk_sb = pp.tile([P, NDT * 512], BF16, name="wqk_sb")
            wv_sb = pp.tile([P, NDT * 256], BF16, name="wv_sb")
            wout_sb = pp.tile([P, CFT * OSL], BF16, name="wout_sb")
            bqk_sb = pp.tile([P, NFT], F32, name="bqk_sb")
            bv_row = pp.tile([1, 256], F32, name="bv_row")
            bvb_sb = pp.tile([P, 256], F32, name="bvb_sb")
            bob_row = pp.tile([1, OSL], F32, name="bob_row")
            bob_sb = pp.tile([P, OSL], F32, name="bob_sb")
            ones_sb = pp.tile([P, DH], BF16, name="ones_sb")
            warm_sb = pp.tile([4, DH], BF16, name="warm_sb")
            qkT = [pp.tile([P, S], BF16, name=f"qkT{ft}") for ft in range(NFT)]
            v_sb = pp.tile([P, (S // P) * 256], BF16, name="v_sb")
            attnT = [pp.tile([P, S], BF16, name=f"attnT{ct}") for ct in range(COT)]
            masks = [pp.tile([P, 2 * QCW], BF16, name=f"mask{j}")
                     for j in range(QCW // P)]

            # ---- DRAM bounce buffers for the AllGathers (bf16),
            # one per (round, head-pair) so each fires mid-round ----
            ag_in = [[dp.tile([P, QCW], BF16, name=f"ag_in{r}_{h}")
                      for h in range(2)] for r in range(NQC)]
            ag_out = [[dp.tile([GW * P, QCW], BF16, name=f"ag_out{r}_{h}")
                       for h in range(2)] for r in range(NQC)]
            warm_in = dp.tile([4, DH], BF16, name="warm_in")
            warm_out = dp.tile([16, DH], BF16, name="warm_out")

            # ---- loads (f32 -> bf16 cast during SWDGE DMA), criticals first
            for ft in range(NFT):
                nc.sync.dma_start(out=bqk_sb[:, ft:ft + 1],
                                  in_=bqk_ext[ft:ft + 1, :].rearrange("o p -> p o"))
            nc.sync.dma_start(out=bv_row[:], in_=bv_ext[:])
            nc.sync.dma_start(out=bob_row[:], in_=bout_ext[:])
            nc.vector.memset(ones_sb[:], 1.0)
            nc.vector.memset(warm_sb[:], 1.0)
            # preload the ACT exp table set before attention needs it
            nc.scalar.activation(warm_sb[0:4, 0:16], warm_sb[0:4, 0:16],
                                 Act.Exp)
            nc.sync.dma_start(out=warm_in[:], in_=warm_sb[:])
            for j in range(QCW // P):
                nc.vector.memset(masks[j][:], 1.0)

            # warm up the collective engine first (cold-start is ~25us and
            # the first trigger also absorbs cross-core start skew)
            warm_cc = nc.gpsimd.collective_compute(
                "AllGather", Alu.bypass, replica_groups=GROUPS,
                ins=[warm_in[:]], outs=[warm_out[:]])
            # everything round 0 needs comes first: wqk, xch0, wv, biases, masks
            xchs = [xp.tile([P, NDT * QCW], BF16, name=f"xch{mc}", tag=f"x{mc}")
                    for mc in range(NMC)]
            hd = NDT // 2
            for half in range(2):
                ds = slice(half * hd, (half + 1) * hd)
                nc.gpsimd.dma_start(
                    out=wqk_sb[:, half * hd * 512:(half + 1) * hd * 512]
                        .rearrange("p (d f) -> p d f", d=hd),
                    in_=wqk_ext[:, ds])
                nc.gpsimd.dma_start(
                    out=xchs[0][:, half * hd * QCW:(half + 1) * hd * QCW]
                        .rearrange("p (d m) -> p d m", d=hd),
                    in_=xt_ext[:, ds, 0:QCW])
            nc.gpsimd.dma_start(
                out=wv_sb[:].rearrange("p (d f) -> p d f", d=NDT),
                in_=wv_ext[:])
            nc.gpsimd.partition_broadcast(bvb_sb[:], bv_row[:])
            nc.gpsimd.partition_broadcast(bob_sb[:], bob_row[:])
            for j in range(QCW // P):
                nc.gpsimd.affine_select(
                    masks[j][:].rearrange("p (s w) -> p s w", s=2),
                    masks[j][:].rearrange("p (s w) -> p s w", s=2),
                    pattern=[[0, 2], [1, QCW]], compare_op=Alu.is_ge,
                    fill=0.0, base=-j * P, channel_multiplier=-1)
            for mc in range(1, NMC):
                nc.gpsimd.dma_start(
                    out=xchs[mc][:].rearrange("p (d m) -> p d m", d=NDT),
                    in_=xt_ext[:, :, mc * QCW:(mc + 1) * QCW])
                if mc == 1:
                    nc.gpsimd.dma_start(
                        out=wout_sb[:].rearrange("p (c f) -> p c f", c=CFT),
                        in_=wout_ext[:])

            # ---- projection work units (one x-chunk = 4 qk + 4 v units) ----
            def do_qk(mc, ft):
                xch = xchs[mc]
                pqk = ps.tile([P, 512], F32, name="pqk", tag="pv", bufs=2)
                for d in range(NDT):
                    nc.tensor.matmul(
                        pqk[:],
                        wqk_sb[:, d * 512 + ft * P:d * 512 + (ft + 1) * P],
                        xch[:, d * QCW:(d + 1) * QCW],
                        start=(d == 0), stop=(d == NDT - 1))
                nc.vector.tensor_scalar_add(
                    qkT[ft][:, mc * QCW:(mc + 1) * QCW], pqk[:],
                    bqk_sb[:, ft:ft + 1])

            def do_v(mc, mt):
                xch = xchs[mc]
                gmt = mc * (QCW // P) + mt
                pv = ps.tile([P, 256], F32, name="pv", tag="pv", bufs=2)
                for d in range(NDT):
                    nc.tensor.matmul(
                        pv[:],
                        xch[:, d * QCW + mt * P:d * QCW + (mt + 1) * P],
                        wv_sb[:, d * 256:(d + 1) * 256],
                        start=(d == 0), stop=(d == NDT - 1))
                nc.vector.tensor_add(v_sb[:, gmt * 256:(gmt + 1) * 256],
                                     pv[:], bvb_sb[:])

            # out-projection of the gathered attention for q-chunk rr:
            # out[q, osl] = sum_c attn_full[q, c] * w_out[c, osl]
            # (c rows arrive pair-split; w_out rows are host-permuted to match)
            af_tiles = {}

            def load_af(rr, h):
                af = afp.tile([P, GW * QCW], BF16, name=f"af{rr}_{h}",
                              tag="af")
                for ct in range(GW):
                    nc.sync.dma_start(
                        out=af[:, ct * QCW:(ct + 1) * QCW],
                        in_=ag_out[rr][h][ct * P:(ct + 1) * P, :])
                af_tiles[(rr, h)] = af

            def do_outproj(rr, qi, pin=None):
                po = ps.tile([P, OSL], F32, name="po", tag="pv", bufs=2)
                for h in range(2):
                    af = af_tiles[(rr, h)]
                    for ct in range(GW):
                        mm = nc.tensor.matmul(
                            po[:],
                            af[:, ct * QCW + qi * P:ct * QCW + (qi + 1) * P],
                            wout_sb[:, (h * GW + ct) * OSL:(h * GW + ct + 1) * OSL],
                            start=(h == 0 and ct == 0),
                            stop=(h == 1 and ct == GW - 1))
                        if pin is not None and h == 0 and ct == 0:
                            # ordering-only pin: keep the scheduler from
                            # hoisting this ahead of the current round's
                            # attention (it would stall PE on the AllGather)
                            add_dep_helper(mm.ins, pin.ins, sync=False,
                                           reason="outproj after attention")
                outsb = osp.tile([P, OSL], BF16, name="outsb", tag="ot")
                nc.vector.tensor_add(outsb[:], po[:], bob_sb[:])
                nc.sync.dma_start(
                    out=out_ext[rr * QCW + qi * P:rr * QCW + (qi + 1) * P, :],
                    in_=outsb[:])

            def proj_units(mc):
                return ([(do_qk, mc, ft) for ft in range(NFT)] +
                        [(do_v, mc, mt) for mt in range(QCW // P)])

            # chunk 0 projection up front
            for fn, a1, a2 in proj_units(0):
                fn(a1, a2)

            # ---- fused rounds ----
            for r in range(NQC):
                units = proj_units(r + 1) if r + 1 < NMC else []
                # out-projection of chunk r-2 (its AllGathers have landed),
                # paced into the last quarter of this round
                ounits = []
                if r >= 2:
                    load_af(r - 2, 0)
                    load_af(r - 2, 1)
                    ounits = [(do_outproj, r - 2, qi)
                              for qi in range(QCW // P)]
                ui = 0
                oi = 0
                last_mm = [None]
                nkt = (r + 1) * (QCW // P)
                steps_total = 2 * nkt
                step = 0
                q0 = r * QCW
                for pr in range(HL // 2):        # head pairs (2pr, 2pr+1)
                    qt_t = qkT[pr]
                    kt_t = qkT[2 + pr]
                    av = ps.tile([P, QCW], F32, name="av", tag="av", bufs=1)
                    den = ps.tile([P, QCW], F32, name="den", tag="den", bufs=1)

                    def av_den(kt, pt2, a, first, last):
                        for sub in (0, 1):
                            h = 2 * pr + sub
                            nc.tensor.matmul(
                                av[sub * DH:(sub + 1) * DH, a:QCW],
                                v_sb[:, kt * 256 + h * DH:kt * 256 + (h + 1) * DH],
                                pt2[:, sub * QCW + a:(sub + 1) * QCW],
                                start=first, stop=last, skip_group_check=True)
                        for sub in (0, 1):
                            nc.tensor.matmul(
                                den[sub * DH:(sub + 1) * DH, a:QCW],
                                ones_sb[:],
                                pt2[:, sub * QCW + a:(sub + 1) * QCW],
                                start=first, stop=last, skip_group_check=True)

                    pending = []
                    for kt in range(nkt):
                        off = max(0, (kt - 4 * r) * P)
                        a = off
                        s2 = ps.tile([P, 2 * QCW], F32, name="s2", tag="s",
                                     bufs=2)
                        for sub in (0, 1):
                            bp = sub * DH
                            last_mm[0] = nc.tensor.matmul(
                                s2[:, sub * QCW + a:(sub + 1) * QCW],
                                kt_t[bp:bp + DH, kt * P:(kt + 1) * P],
                                qt_t[bp:bp + DH, q0 + a:q0 + QCW],
                                start=True, stop=True)
                        pt2 = ptp.tile([P, 2 * QCW], BF16, name="pt2", tag="pt")
                        s2v = s2[:].rearrange("p (s w) -> p s w", s=2)
                        pt2v = pt2[:].rearrange("p (s w) -> p s w", s=2)
                        nc.scalar.activation(pt2v[:, :, a:QCW], s2v[:, :, a:QCW],
                                             Act.Exp)
                        if kt >= 4 * r:
                            j = kt - 4 * r
                            wlen = min(off + P, QCW) - a
                            mv = masks[j][:].rearrange("p (s w) -> p s w", s=2)
                            nc.vector.tensor_mul(pt2v[:, :, a:a + wlen],
                                                 pt2v[:, :, a:a + wlen],
                                                 mv[:, :, a:a + wlen])
                        # pace projection units across the round, out-proj
                        # units into its last quarter
                        if units and ui < len(units) and \
                                step * len(units) >= ui * steps_total:
                            fn, a1, a2 = units[ui]
                            fn(a1, a2)
                            ui += 1
                        q3 = 3 * steps_total // 4
                        if ounits and oi < len(ounits) and step >= q3 and \
                                (step - q3) * len(ounits) >= \
                                oi * (steps_total - q3):
                            fn, a1, a2 = ounits[oi]
                            fn(a1, a2, pin=last_mm[0])
                            oi += 1
                        pending.append((kt, pt2, a))
                        if len(pending) > 2:
                            pv_ = pending.pop(0)
                            av_den(*pv_, first=(pv_[0] == 0), last=False)
                        step += 1
                    while pending:
                        pv_ = pending.pop(0)
                        av_den(*pv_, first=(pv_[0] == 0),
                               last=(not pending))

                    recip = rcp.tile([P, QCW], F32, name="recip", tag="rc")
                    nc.vector.reciprocal_approx_fast(recip[:], den[:])
                    nc.vector.tensor_mul(attnT[pr][:, q0:q0 + QCW], av[:],
                                         recip[:])
                    # ship this pair's attention and AllGather it right away
                    nc.scalar.dma_start(out=ag_in[r][pr][:],
                                        in_=attnT[pr][:, q0:q0 + QCW])
                    nc.gpsimd.collective_compute(
                        "AllGather", Alu.bypass, replica_groups=GROUPS,
                        ins=[ag_in[r][pr][:]], outs=[ag_out[r][pr][:]])
                while ui < len(units):
                    fn, a1, a2 = units[ui]
                    fn(a1, a2)
                    ui += 1
                while oi < len(ounits):
                    fn, a1, a2 = ounits[oi]
                    fn(a1, a2)
                    oi += 1

            # tail: chunk NQC-2's out-projection (AllGather already landed,
            # fills the wait for the final AllGather), then the last chunk's
            for rr in (NQC - 2, NQC - 1):
                load_af(rr, 0)
                load_af(rr, 1)
                for qi in range(QCW // P):
                    do_outproj(rr, qi)

    nc.compile()
    return nc


def shard_inputs(x, w_qkv, b_qkv, w_out, b_out, S=S_FULL):
    """Host-side sharding: per-core input dicts (pure layout work)."""
    scale = np.float32(DH ** -0.5)
    x = np.asarray(x, dtype=np.float32)
    w_qkv = np.asarray(w_qkv, dtype=np.float32)
    b_qkv = np.asarray(b_qkv, dtype=np.float32)
    w_out = np.asarray(w_out, dtype=np.float32)
    b_out = np.asarray(b_out, dtype=np.float32)
    NDT = D // P
    CFT = H * DH // P
    in_maps = []
    for c in range(NCORE):
        g, r = c // 4, c % 4
        hs = slice(HL * r, HL * (r + 1))
        osl = slice(OSL * r, OSL * (r + 1))
        xt = np.ascontiguousarray(
            x[g, :S].T.reshape(NDT, P, S).transpose(1, 0, 2))
        wq = (w_qkv[:, hs, 0:DH] * scale).reshape(D, HL * DH)
        wk = w_qkv[:, hs, DH:2 * DH].reshape(D, HL * DH)
        wqk = np.ascontiguousarray(
            np.concatenate([wq, wk], axis=1).reshape(NDT, P, 2 * HL * DH)
            .transpose(1, 0, 2))
        wv = np.ascontiguousarray(
            w_qkv[:, hs, 2 * DH:3 * DH].reshape(D, HL * DH)
            .reshape(NDT, P, HL * DH).transpose(1, 0, 2))
        head_order = ([4 * rr + j for rr in range(4) for j in (0, 1)] +
                      [4 * rr + j for rr in range(4) for j in (2, 3)])
        wo = np.ascontiguousarray(
            w_out[head_order].reshape(H * DH, D)[:, osl]
            .reshape(CFT, P, OSL).transpose(1, 0, 2))
        bq = (b_qkv[hs, 0:DH] * scale).reshape(HL * DH)
        bk = b_qkv[hs, DH:2 * DH].reshape(HL * DH)
        bqk = np.concatenate([bq, bk]).reshape(2 * HL * DH // P, P)
        bv = b_qkv[hs, 2 * DH:3 * DH].reshape(1, HL * DH)
        bout = b_out[osl].reshape(1, OSL)
        in_maps.append({
            "xt": np.ascontiguousarray(xt),
            "wqk": wqk, "wv": wv, "wout": wo,
            "bqk": np.ascontiguousarray(bqk),
            "bv": np.ascontiguousarray(bv),
            "bout": np.ascontiguousarray(bout),
        })
    return in_maps


def unshard_output(results, S=S_FULL):
    """Pure gather of per-core column slices into the full output (+cast)."""
    out = np.empty((BS, S, D), dtype=np.float32)
    for c in range(NCORE):
        g, r = c // 4, c % 4
        o = np.asarray(results[c]["out"]).astype(np.float32)
        out[g, :, OSL * r:OSL * (r + 1)] = o
    return out


def kernel(x, w_qkv, b_qkv, w_out, b_out, trace=False):
    from concourse.bass_utils import run_bass_kernel_spmd
    if "nc" not in _CACHE:
        _CACHE["nc"] = build_graph()
    nc = _CACHE["nc"]
    in_maps = shard_inputs(x, w_qkv, b_qkv, w_out, b_out)
    res = run_bass_kernel_spmd(nc, in_maps, core_ids=list(range(NCORE)),
                               trace=trace)
    _CACHE["last_results"] = res
    return unshard_output(res.results)
